# revision 1
# baseline (speedup 1.0000x reference)
"""Distributed kNN retrieval kernel for trn2 (8 NeuronCores).

Math: reference ranks candidates per query by cosine distance
1 - dot/(|q||m|); query norm is constant per row, so ranking is by
dot(q, m)/|m|.  Host pre-normalizes matching rows (fp64 norms), so the
device only computes S = Q @ Mn^T, takes per-query top-4, and averages
synth rows.

Distribution: candidates (100000) row-sharded 12500/core;
synth column-sharded 128 features/core.  Per core:
  bf16x3 matmul (Qh.Mh + Qh.Ml + Ql.Mh accumulated in fp32 PSUM)
  -> per-block top-8 (nc.vector.max/max_index) -> local top-4
  -> AllGather (2048 x 8 fp32) -> replicated global top-4 merge
  -> indirect-DMA gather of this core's 128 synth columns -> mean
  -> output [2048, 128] slice; host concatenates along features.
"""
import sys

import numpy as np

sys.path.insert(0, "/opt/trn_rl_repo")
import ml_dtypes  # noqa: E402
import concourse.bacc as bacc  # noqa: E402
import concourse.bass as bass  # noqa: E402
import concourse.mybir as mybir  # noqa: E402
import concourse.tile as tile  # noqa: E402
from concourse.bass import IndirectOffsetOnAxis  # noqa: E402
from concourse.bass_utils import run_bass_kernel_spmd  # noqa: E402

NCORES = 8
FRM = 2048          # queries
F = 1024            # features
C = 100000          # candidates
SHARD = C // NCORES         # 12500
CW = 500                    # candidate-chunk width (25*500 = 12500, no pad)
KCH = F // 128              # 8 contraction chunks
NQT = FRM // 128            # 16 query tiles
NCCH = SHARD // CW          # 25 candidate chunks
BLOCKS = [(b * 3, 3) for b in range(8)] + [(24, 1)]  # (cchunk0, n) -> 9 blocks
NB = len(BLOCKS)
NCAND = NB * 8              # 72 local candidates per query
FSL = F // NCORES           # 128 synth feature columns per core
SPLITS = [(0, 4), (4, 4), (8, 5), (13, 3)]  # (start, n) qtile groups

BF16 = mybir.dt.bfloat16
F32 = mybir.dt.float32
U32 = mybir.dt.uint32
I32 = mybir.dt.int32


def build():
    nc = bacc.Bacc(num_devices=NCORES)
    QHL = nc.declare_dram_parameter("qhl", [128, 2 * KCH * FRM], BF16, isOutput=False)
    MHL = nc.declare_dram_parameter("mhl", [NCCH, 128, KCH * 2 * CW], BF16, isOutput=False)
    SYN = nc.declare_dram_parameter("syn", [C, FSL], F32, isOutput=False)
    COFF = nc.declare_dram_parameter("coff", [128, 1], F32, isOutput=False)
    OUT = nc.declare_dram_parameter("out", [FRM, FSL], F32, isOutput=True)

    HI_OFF = KCH * FRM  # bf16 column offset of the lo half in QHL

    with tile.TileContext(nc) as tc:
        with tc.tile_pool(name="cst", bufs=1) as cst, \
             tc.tile_pool(name="mpool", bufs=6) as mpool, \
             tc.tile_pool(name="blk", bufs=3) as blk, \
             tc.tile_pool(name="cand", bufs=2) as cand, \
             tc.tile_pool(name="sm", bufs=8) as sm, \
             tc.tile_pool(name="gat", bufs=6) as gat, \
             tc.tile_pool(name="psw", bufs=1, space="PSUM") as psw, \
             tc.tile_pool(name="ps", bufs=6, space="PSUM") as ps, \
             tc.tile_pool(name="dram", bufs=4, space="DRAM") as dram:

            # tiny warmup weights + first-qtile weights first, so PE and the
            # first matmul group start as soon as possible
            QB = KCH * 128  # 1024 cols per qtile block
            wt = cst.tile([128, 128], BF16)
            nc.sync.dma_start(out=wt[:], in_=QHL[:, :128])
            qhl = cst.tile([128, 2 * KCH * FRM], BF16)
            nc.sync.dma_start(out=qhl[:, :QB], in_=QHL[:, :QB])
            nc.sync.dma_start(out=qhl[:, HI_OFF:HI_OFF + QB],
                              in_=QHL[:, HI_OFF:HI_OFF + QB])
            coff = cst.tile([128, 1], F32)
            nc.sync.dma_start(out=coff[:], in_=COFF[:])

            pw = psw.tile([128, 128], F32)
            nc.tensor.matmul(out=pw[:], lhsT=wt[:], rhs=wt[:],
                             start=True, stop=True)

            def qs(hl, k, t):
                base = hl * HI_OFF + t * KCH * 128 + k * 128
                return qhl[:, base:base + 128]

            for s, (q0, QTPS) in enumerate(SPLITS):
                qtiles = range(q0, q0 + QTPS)
                valsall = cand.tile([128, 5 * NCAND], F32, tag="valsall")
                idxall = cand.tile([128, 5 * NCAND], F32, tag="idxall")

                # ---- phase 1+2: scores + per-block top-8 ----
                for b, (c0, nch) in enumerate(BLOCKS):
                    mts = []
                    for ci in range(nch):
                        mt = mpool.tile([128, KCH * 2 * CW], BF16, tag="mt")
                        nc.sync.dma_start(out=mt[:], in_=MHL[c0 + ci])
                        mts.append(mt)
                    if s == 0 and b == 0:
                        # rest of the Q weights, after the urgent first tiles
                        nc.sync.dma_start(out=qhl[:, QB:HI_OFF],
                                          in_=QHL[:, QB:HI_OFF])
                        nc.sync.dma_start(out=qhl[:, HI_OFF + QB:],
                                          in_=QHL[:, HI_OFF + QB:])
                    for tl, t in enumerate(qtiles):
                        sblk = blk.tile([128, 3 * CW], F32, tag="sblk")
                        for ci in range(nch):
                            p = ps.tile([128, CW], F32, tag="p")
                            i = 0
                            for hq, hm in ((0, 0), (0, 1), (1, 0)):
                                for k in range(KCH):
                                    ms = mts[ci][:, (k * 2 + hm) * CW:
                                                 (k * 2 + hm + 1) * CW]
                                    nc.tensor.matmul(out=p[:], lhsT=qs(hq, k, t),
                                                     rhs=ms,
                                                     start=(i == 0), stop=(i == 23))
                                    i += 1
                            nc.scalar.copy(out=sblk[:, ci * CW:(ci + 1) * CW], in_=p[:])
                        sb_v = sblk[:, :nch * CW]
                        vsl = valsall[:, tl * NCAND + b * 8: tl * NCAND + b * 8 + 8]
                        isl = idxall[:, tl * NCAND + b * 8: tl * NCAND + b * 8 + 8]
                        bi = sm.tile([128, 8], U32, tag="bi")
                        bif = sm.tile([128, 8], F32, tag="bif")
                        nc.vector.max(out=vsl, in_=sb_v)
                        nc.vector.max_index(out=bi[:], in_max=vsl, in_values=sb_v)
                        nc.vector.tensor_copy(out=bif[:], in_=bi[:])  # u32 -> f32
                        # global candidate id = pos + CW*c0 + core_offset
                        nc.vector.tensor_scalar(
                            out=isl, in0=bif[:], scalar1=coff[:, 0:1],
                            scalar2=float(CW * c0),
                            op0=mybir.AluOpType.add, op1=mybir.AluOpType.add)

                # ---- phase 2b: local top-4 -> cc_in ----
                cc_in = dram.tile([QTPS * 128, 8], F32, tag="ccin")
                cc_out = dram.tile([NCORES * QTPS * 128, 8], F32, tag="ccout")
                for tl, t in enumerate(qtiles):
                    va = valsall[:, tl * NCAND:(tl + 1) * NCAND]
                    ia = idxall[:, tl * NCAND:(tl + 1) * NCAND]
                    lv = sm.tile([128, 8], F32, tag="lv")
                    loc = sm.tile([128, 8], F32, tag="loc")
                    nc.vector.max(out=lv[:], in_=va)
                    nc.vector.tensor_copy(out=loc[:, 0:4], in_=lv[:, 0:4])
                    eq = sm.tile([128, 4 * NCAND], F32, tag="eq")
                    eq3 = eq[:].rearrange("p (j n) -> p j n", j=4)
                    nc.vector.tensor_tensor(
                        out=eq3, in0=va.unsqueeze(1).to_broadcast([128, 4, NCAND]),
                        in1=lv[:, 0:4].unsqueeze(2).to_broadcast([128, 4, NCAND]),
                        op=mybir.AluOpType.is_equal)
                    nc.vector.tensor_tensor(
                        out=eq3, in0=eq3,
                        in1=ia.unsqueeze(1).to_broadcast([128, 4, NCAND]),
                        op=mybir.AluOpType.mult)
                    nc.vector.tensor_reduce(
                        out=loc[:, 4:8], in_=eq3,
                        axis=mybir.AxisListType.X, op=mybir.AluOpType.max)
                    nc.sync.dma_start(out=cc_in[tl * 128:(tl + 1) * 128, :], in_=loc[:])

                # ---- phase 3: AllGather candidates for this split ----
                nc.gpsimd.collective_compute(
                    "AllGather", mybir.AluOpType.bypass,
                    replica_groups=[list(range(NCORES))],
                    ins=[cc_in.opt()], outs=[cc_out.opt()])

                # ---- phase 4+5: global merge, gather-accumulate, mean ----
                cc_view = cc_out[:].rearrange("(r q) e -> q r e", r=NCORES)
                for tl, t in enumerate(qtiles):
                    cands = sm.tile([128, NCORES * 8], F32, tag="cands")
                    nc.sync.dma_start(
                        out=cands[:].rearrange("p (r e) -> p r e", r=NCORES),
                        in_=cc_view[tl * 128:(tl + 1) * 128])
                    cv = sm.tile([128, 32], F32, tag="cv")
                    cvi = sm.tile([128, 32], F32, tag="cvi")
                    c3 = cands[:].rearrange("p (r e) -> p r e", r=NCORES)
                    nc.vector.tensor_copy(out=cv[:].rearrange("p (r e) -> p r e", r=8),
                                          in_=c3[:, :, 0:4])
                    nc.vector.tensor_copy(out=cvi[:].rearrange("p (r e) -> p r e", r=8),
                                          in_=c3[:, :, 4:8])
                    gv = sm.tile([128, 8], F32, tag="gv")
                    gif = sm.tile([128, 4], F32, tag="gif")
                    nc.vector.max(out=gv[:], in_=cv[:])
                    eq2 = sm.tile([128, 4 * 32], F32, tag="eq2")
                    e3 = eq2[:].rearrange("p (j n) -> p j n", j=4)
                    nc.vector.tensor_tensor(
                        out=e3, in0=cv[:].unsqueeze(1).to_broadcast([128, 4, 32]),
                        in1=gv[:, 0:4].unsqueeze(2).to_broadcast([128, 4, 32]),
                        op=mybir.AluOpType.is_equal)
                    nc.vector.tensor_tensor(
                        out=e3, in0=e3,
                        in1=cvi[:].unsqueeze(1).to_broadcast([128, 4, 32]),
                        op=mybir.AluOpType.mult)
                    nc.vector.tensor_reduce(
                        out=gif[:], in_=e3,
                        axis=mybir.AxisListType.X, op=mybir.AluOpType.max)
                    gii = sm.tile([128, 4], I32, tag="gii")
                    nc.vector.tensor_copy(out=gii[:], in_=gif[:])  # f32 -> i32
                    gbuf = gat.tile([128, FSL], F32, tag="gbuf")
                    nc.vector.memset(gbuf[:], 0.0)
                    for j in range(4):
                        nc.gpsimd.indirect_dma_start(
                            out=gbuf[:], out_offset=None,
                            in_=SYN[:],
                            in_offset=IndirectOffsetOnAxis(ap=gii[:, j:j + 1], axis=0),
                            compute_op=mybir.AluOpType.add)
                    nc.vector.tensor_scalar_mul(gbuf[:], gbuf[:], 0.25)
                    nc.sync.dma_start(out=OUT[t * 128:(t + 1) * 128, :], in_=gbuf[:])

    nc.compile()
    return nc


# ---------------- host side ----------------

def _split_bf16(x):
    hi = x.astype(ml_dtypes.bfloat16)
    lo = (x - hi.astype(np.float32)).astype(ml_dtypes.bfloat16)
    return hi, lo


def prepare_inputs(query_seq, matching_set, synth_set):
    """Returns per-core in_maps."""
    q = np.asarray(query_seq, dtype=np.float32)
    m = np.asarray(matching_set, dtype=np.float32)
    syn = np.asarray(synth_set, dtype=np.float32)

    # normalize matching rows with fp64 norms
    norms = np.linalg.norm(m.astype(np.float64), axis=1, keepdims=True)
    mn = (m / norms).astype(np.float32)

    # Q^T packed [128, 2*KCH*FRM]
    qt = np.ascontiguousarray(q.T)                       # [1024, 2048]
    qh, ql = _split_bf16(qt)
    def pack_q(a):
        return a.reshape(KCH, 128, NQT, 128).transpose(1, 2, 0, 3).reshape(128, KCH * FRM)
    qhl = np.concatenate([pack_q(qh), pack_q(ql)], axis=1).copy()

    in_maps = []
    for core in range(NCORES):
        shard = mn[core * SHARD:(core + 1) * SHARD]      # [12500, 1024]
        mt = np.ascontiguousarray(shard.T)               # [1024, 12500]
        mh, ml = _split_bf16(mt)
        # [k,p,c,n] -> [c,p,k,hl,n] -> [25, 128, 8000]
        mh4 = mh.reshape(KCH, 128, NCCH, CW).transpose(2, 1, 0, 3)
        ml4 = ml.reshape(KCH, 128, NCCH, CW).transpose(2, 1, 0, 3)
        mhl = np.stack([mh4, ml4], axis=3).reshape(NCCH, 128, KCH * 2 * CW).copy()

        in_maps.append({
            "qhl": qhl,
            "mhl": mhl,
            "syn": np.ascontiguousarray(syn[:, core * FSL:(core + 1) * FSL]),
            "coff": np.full((128, 1), float(core * SHARD), dtype=np.float32),
        })
    return in_maps


_NC_CACHE = {}


def run(query_seq, matching_set, synth_set, topk=4, trace=False):
    assert int(topk) == 4, f"kernel is specialized for topk=4, got {topk}"
    in_maps = prepare_inputs(query_seq, matching_set, synth_set)
    if "nc" not in _NC_CACHE:
        _NC_CACHE["nc"] = build()
    nc = _NC_CACHE["nc"]
    res = run_bass_kernel_spmd(nc, in_maps, core_ids=list(range(NCORES)),
                               trace=trace)
    out = np.concatenate([res.results[i]["out"] for i in range(NCORES)], axis=1)
    return out.astype(np.float32), res


def kernel(**inputs):
    topk = inputs.get("topk", 4)
    try:
        topk = int(np.asarray(topk))
    except Exception:
        topk = int(topk)
    out, _ = run(inputs["query_seq"], inputs["matching_set"],
                 inputs["synth_set"], topk)
    return out



# revision 5
# speedup vs baseline: 2.7102x; 2.7102x over previous
"""Distributed kNN retrieval kernel for trn2 (8 NeuronCores) — v2.

Math: reference ranks candidates per query by cosine distance
1 - dot/(|q||m|); the query norm is constant per row, so ranking is by
dot(q, m)/|m|.  Host pre-normalizes matching rows (fp64 norms).

Two-stage select-then-rerank:
  stage 1 (approx): fp8-e4m3 DoubleRow matmul (2x bf16 rate) computes
    all 2048 x 12500 scores per core (candidates row-sharded).  Per-core
    top-8 per query via DVE max8/find_index8 on bf16 score rows.
  AllToAll: query-shards the 8x top-8 candidate lists (core c receives
    all peers' candidates for queries it owns).
  stage 2 (exact): merge to global top-16 per query, gather those 16
    matching rows (f32) by indirect DMA, recompute exact f32 dots on
    DVE, take top-4.
  AllGather final ids; every core gathers its 128 synth feature columns
  (synth feature-sharded) and averages; host concatenates features.

fp8 precision safety: e4m3 roundoff gives score-error sigma ~0.05 in
unit-score terms while the rank-4 -> rank-16 margin is ~0.34 (5 sigma),
so the true top-4 lies in the approx global top-16 w.h.p.; the exact
f32 rerank then reproduces the reference ranking.
"""
import sys

import numpy as np

sys.path.insert(0, "/opt/trn_rl_repo")
import ml_dtypes  # noqa: E402
import concourse.bacc as bacc  # noqa: E402
import concourse.bass as bass  # noqa: E402
import concourse.mybir as mybir  # noqa: E402
import concourse.tile as tile  # noqa: E402
from concourse.bass import IndirectOffsetOnAxis  # noqa: E402
from concourse.bass_utils import run_bass_kernel_spmd  # noqa: E402

NCORES = 8
FRM = 2048          # queries
F = 1024            # features
C = 100000          # candidates
SHARD = C // NCORES         # 12500
CW = 512                    # candidate-chunk width
NCCH = 25                   # chunks per core
SHARD_PAD = CW * NCCH       # 12800 (300 zero-pad rows)
NKP = 4                     # fp8 DoubleRow k-pairs (4 x 256 = 1024)
NQT = FRM // 128            # 16 query tiles
GQT = 4                     # qtiles per m8 sweep group
SQ = 16.0                   # fp8 scale for q
SM = 64.0                   # fp8 scale for normalized m
NSEL = 16                   # global rerank candidates per query

BF16 = mybir.dt.bfloat16
F32 = mybir.dt.float32
F8 = mybir.dt.float8e4
U16 = mybir.dt.uint16
U32 = mybir.dt.uint32
I32 = mybir.dt.int32
AOP = mybir.AluOpType


def build():
    nc = bacc.Bacc(num_devices=NCORES)
    Q8 = nc.declare_dram_parameter("q8", [128, NKP * 2 * FRM], F8, isOutput=False)
    M8 = nc.declare_dram_parameter("m8", [NCCH, 128, NKP * 2 * CW], F8, isOutput=False)
    QF = nc.declare_dram_parameter("qf", [256, F], F32, isOutput=False)
    MF = nc.declare_dram_parameter("mf", [C, F], F32, isOutput=False)
    SYN = nc.declare_dram_parameter("syn", [C, 128], F32, isOutput=False)
    COFF = nc.declare_dram_parameter("coff", [128, 1], F32, isOutput=False)
    OFFS = nc.declare_dram_parameter("offs", [128, 64], F32, isOutput=False)
    OUT = nc.declare_dram_parameter("out", [FRM, 128], F32, isOutput=True)

    with tile.TileContext(nc) as tc:
        with tc.tile_pool(name="cst", bufs=1) as cst, \
             tc.tile_pool(name="mpool", bufs=3) as mpool, \
             tc.tile_pool(name="spool", bufs=5) as spool, \
             tc.tile_pool(name="sm", bufs=8) as sm, \
             tc.tile_pool(name="rk", bufs=2) as rk, \
             tc.tile_pool(name="gat", bufs=4) as gat, \
             tc.tile_pool(name="sy", bufs=4) as sy, \
             tc.tile_pool(name="psw", bufs=1, space="PSUM") as psw, \
             tc.tile_pool(name="ps", bufs=4, space="PSUM") as ps, \
             tc.tile_pool(name="dram", bufs=2, space="DRAM") as dram:

            # ---- early loads + PE warmup ----
            q8t = cst.tile([128, NKP, 2, FRM], F8)
            nc.sync.dma_start(out=q8t[:, 0], in_=Q8[:, 0:2 * FRM])
            coff = cst.tile([128, 1], F32)
            nc.sync.dma_start(out=coff[:], in_=COFF[:])
            offs = cst.tile([128, 64], F32)
            nc.sync.dma_start(out=offs[:], in_=OFFS[:])

            pw = psw.tile([128, 128], F32)
            nc.tensor.matmul(out=pw[:], lhsT=q8t[:, 0, 0, 0:128],
                             rhs=q8t[:, 0, 0, 0:128], start=True, stop=True)
            for kp in range(1, NKP):
                nc.sync.dma_start(out=q8t[:, kp],
                                  in_=Q8[:, kp * 2 * FRM:(kp + 1) * 2 * FRM])

            at_in = [dram.tile([8 * 128, 16], F32, name=f"atin{h}", tag=f"atin{h}") for h in range(2)]
            at_out = [dram.tile([8 * 128, 16], F32, name=f"atout{h}", tag=f"atout{h}") for h in range(2)]
            ag_in = [dram.tile([128, 4], F32, name=f"agin{h}", tag=f"agin{h}") for h in range(2)]
            ag_out = [dram.tile([8 * 128, 4], F32, name=f"agout{h}", tag=f"agout{h}") for h in range(2)]

            def stage1_qtile_group(g):
                """Scores + per-core top-8 for qtiles 4g..4g+3."""
                srows = [spool.tile([128, SHARD_PAD], BF16, name=f"srow{g}_{i}", tag="srow")
                         for i in range(GQT)]
                for ch in range(NCCH):
                    mt = mpool.tile([128, NKP, 2, CW], F8, tag="mt")
                    nc.sync.dma_start(out=mt[:], in_=M8[ch])
                    for tl in range(GQT):
                        t = GQT * g + tl
                        p = ps.tile([128, CW], F32, tag="p")
                        for kp in range(NKP):
                            nc.tensor.matmul(
                                out=p[:],
                                lhsT=q8t[:, kp, :, t * 128:(t + 1) * 128],
                                rhs=mt[:, kp],
                                start=(kp == 0), stop=(kp == NKP - 1),
                                perf_mode=mybir.MatmulPerfMode.DoubleRow)
                        nc.scalar.copy(out=srows[tl][:, ch * CW:(ch + 1) * CW],
                                       in_=p[:])
                for tl in range(GQT):
                    t = GQT * g + tl
                    v8 = sm.tile([128, 8], BF16, tag="v8")
                    nc.vector.max(out=v8[:], in_=srows[tl][:])
                    i8 = sm.tile([128, 8], U16, tag="i8")
                    nc.vector.max_index(out=i8[:], in_max=v8[:], in_values=srows[tl][:])
                    pay = sm.tile([128, 16], F32, tag="pay")
                    nc.vector.tensor_copy(out=pay[:, 0:8], in_=v8[:])
                    i8f = sm.tile([128, 8], F32, tag="i8f")
                    nc.vector.tensor_copy(out=i8f[:], in_=i8[:])
                    nc.vector.tensor_scalar(
                        out=pay[:, 8:16], in0=i8f[:], scalar1=coff[:, 0:1],
                        scalar2=0.0, op0=AOP.add, op1=AOP.add)
                    h, tt = divmod(t, 8)
                    nc.sync.dma_start(out=at_in[h][tt * 128:(tt + 1) * 128, :],
                                      in_=pay[:])

            def rerank_block(h):
                """Merge + exact rerank for this core's qtile of half h."""
                cand = rk.tile([128, 8, 16], F32, tag="cand")
                nc.sync.dma_start(
                    out=cand[:],
                    in_=at_out[h][:].rearrange("(r p) e -> p r e", r=8))
                cv = rk.tile([128, 64], F32, tag="cv")
                ci = rk.tile([128, 64], F32, tag="ci")
                nc.vector.tensor_copy(
                    out=cv[:].rearrange("p (r e) -> p r e", r=8),
                    in_=cand[:, :, 0:8])
                nc.vector.tensor_copy(
                    out=ci[:].rearrange("p (r e) -> p r e", r=8),
                    in_=cand[:, :, 8:16])
                # de-tie: unique per-(core,slot) epsilon keeps the eq-trick
                # id resolution collision-free across bf16 score ties
                nc.vector.tensor_tensor(out=cv[:], in0=cv[:], in1=offs[:],
                                        op=AOP.add)
                gids = rk.tile([128, NSEL], F32, tag="gids")
                gv = rk.tile([128, 8], F32, tag="gv")
                cvs = cv
                for rnd in range(NSEL // 8):
                    nc.vector.max(out=gv[:], in_=cvs[:])
                    e3 = rk.tile([128, 8, 64], F32, tag="e3")
                    nc.vector.tensor_tensor(
                        out=e3[:],
                        in0=cvs[:].unsqueeze(1).to_broadcast([128, 8, 64]),
                        in1=gv[:].unsqueeze(2).to_broadcast([128, 8, 64]),
                        op=AOP.is_equal)
                    nc.vector.tensor_tensor(
                        out=e3[:], in0=e3[:],
                        in1=ci[:].unsqueeze(1).to_broadcast([128, 8, 64]),
                        op=AOP.mult)
                    nc.vector.tensor_reduce(
                        out=gids[:, rnd * 8:(rnd + 1) * 8], in_=e3[:],
                        axis=mybir.AxisListType.X, op=AOP.max)
                    if rnd == 0:
                        cv2 = rk.tile([128, 64], F32, tag="cv2")
                        nc.vector.match_replace(
                            out=cv2[:], in_to_replace=gv[:], in_values=cvs[:],
                            imm_value=-1e30)
                        cvs = cv2
                # clamp ids to [0, C-1] then cast to i32
                gcl = rk.tile([128, NSEL], F32, tag="gcl")
                nc.vector.tensor_scalar(
                    out=gcl[:], in0=gids[:], scalar1=float(C - 1), scalar2=0.0,
                    op0=AOP.min, op1=AOP.max)
                gii = rk.tile([128, NSEL], I32, tag="gii")
                nc.vector.tensor_copy(out=gii[:], in_=gcl[:])

                qft = rk.tile([128, F], F32, tag="qft")
                nc.sync.dma_start(out=qft[:], in_=QF[h * 128:(h + 1) * 128, :])
                rs = rk.tile([128, NSEL], F32, tag="rs")
                for j in range(NSEL):
                    gj = gat.tile([128, F], F32, tag="gj")
                    nc.gpsimd.indirect_dma_start(
                        out=gj[:], out_offset=None, in_=MF[:],
                        in_offset=IndirectOffsetOnAxis(ap=gii[:, j:j + 1], axis=0))
                    scr = gat.tile([128, F], F32, tag="scr")
                    nc.vector.scalar_tensor_tensor(
                        out=scr[:], in0=qft[:], scalar=1.0, in1=gj[:],
                        op0=AOP.mult, op1=AOP.mult, accum_out=rs[:, j:j + 1])
                fv = rk.tile([128, 8], F32, tag="fv")
                nc.vector.max(out=fv[:], in_=rs[:])
                e4 = rk.tile([128, 4, NSEL], F32, tag="e4")
                nc.vector.tensor_tensor(
                    out=e4[:],
                    in0=rs[:].unsqueeze(1).to_broadcast([128, 4, NSEL]),
                    in1=fv[:, 0:4].unsqueeze(2).to_broadcast([128, 4, NSEL]),
                    op=AOP.is_equal)
                nc.vector.tensor_tensor(
                    out=e4[:], in0=e4[:],
                    in1=gids[:].unsqueeze(1).to_broadcast([128, 4, NSEL]),
                    op=AOP.mult)
                fi = rk.tile([128, 4], F32, tag="fi")
                nc.vector.tensor_reduce(
                    out=fi[:], in_=e4[:], axis=mybir.AxisListType.X, op=AOP.max)
                nc.sync.dma_start(out=ag_in[h][:], in_=fi[:])

            def synth_block(h):
                """Gather-mean of this core's 128 synth columns, qtiles 8h..8h+7."""
                for r in range(8):
                    t = 8 * h + r
                    idt = sy.tile([128, 4], F32, tag="idt")
                    nc.sync.dma_start(out=idt[:],
                                      in_=ag_out[h][r * 128:(r + 1) * 128, :])
                    idc = sy.tile([128, 4], F32, tag="idc")
                    nc.vector.tensor_scalar(
                        out=idc[:], in0=idt[:], scalar1=float(C - 1), scalar2=0.0,
                        op0=AOP.min, op1=AOP.max)
                    idi = sy.tile([128, 4], I32, tag="idi")
                    nc.vector.tensor_copy(out=idi[:], in_=idc[:])
                    gb = sy.tile([128, 128], F32, tag="gb")
                    nc.vector.memset(gb[:], 0.0)
                    for j in range(4):
                        nc.gpsimd.indirect_dma_start(
                            out=gb[:], out_offset=None, in_=SYN[:],
                            in_offset=IndirectOffsetOnAxis(ap=idi[:, j:j + 1], axis=0),
                            compute_op=AOP.add)
                    nc.vector.tensor_scalar_mul(gb[:], gb[:], 0.25)
                    nc.sync.dma_start(out=OUT[t * 128:(t + 1) * 128, :], in_=gb[:])

            rg = [list(range(NCORES))]
            # ---- half 1: qtiles 0..7 ----
            stage1_qtile_group(0)
            stage1_qtile_group(1)
            nc.gpsimd.collective_compute(
                "AllToAll", AOP.bypass, replica_groups=rg,
                ins=[at_in[0].opt()], outs=[at_out[0].opt()])
            rerank_block(0)
            nc.gpsimd.collective_compute(
                "AllGather", AOP.bypass, replica_groups=rg,
                ins=[ag_in[0].opt()], outs=[ag_out[0].opt()])
            # ---- half 2: qtiles 8..15 (overlaps with rerank/synth of half 1) ----
            stage1_qtile_group(2)
            stage1_qtile_group(3)
            synth_block(0)
            nc.gpsimd.collective_compute(
                "AllToAll", AOP.bypass, replica_groups=rg,
                ins=[at_in[1].opt()], outs=[at_out[1].opt()])
            rerank_block(1)
            nc.gpsimd.collective_compute(
                "AllGather", AOP.bypass, replica_groups=rg,
                ins=[ag_in[1].opt()], outs=[ag_out[1].opt()])
            synth_block(1)

    nc.compile()
    return nc


# ---------------- host side ----------------

def _to_fp8(x):
    return np.clip(x, -240.0, 240.0).astype(ml_dtypes.float8_e4m3)


def prepare_inputs(query_seq, matching_set, synth_set):
    q = np.asarray(query_seq, dtype=np.float32)
    m = np.asarray(matching_set, dtype=np.float32)
    syn = np.asarray(synth_set, dtype=np.float32)

    norms = np.linalg.norm(m.astype(np.float64), axis=1, keepdims=True)
    mn = (m / norms).astype(np.float32)

    # q8: [1024, 2048] -> [kp 4, ko 2, p 128, col 2048] -> [p, kp, ko, col]
    qt = np.ascontiguousarray(q.T) * SQ
    q8 = _to_fp8(qt).reshape(NKP, 2, 128, FRM).transpose(2, 0, 1, 3)
    q8 = np.ascontiguousarray(q8).reshape(128, NKP * 2 * FRM)

    in_maps = []
    for core in range(NCORES):
        shard = mn[core * SHARD:(core + 1) * SHARD] * SM    # [12500, 1024]
        sp = np.zeros((SHARD_PAD, F), dtype=np.float32)
        sp[:SHARD] = shard
        # [1024, 12800] -> [kp, ko, p, ch, cw] -> [ch, p, kp, ko, cw]
        mt8 = _to_fp8(sp.T).reshape(NKP, 2, 128, NCCH, CW).transpose(3, 2, 0, 1, 4)
        mt8 = np.ascontiguousarray(mt8).reshape(NCCH, 128, NKP * 2 * CW)

        qf = np.concatenate([q[core * 128:(core + 1) * 128],
                             q[1024 + core * 128:1024 + (core + 1) * 128]], axis=0)

        in_maps.append({
            "q8": q8,
            "m8": mt8,
            "qf": np.ascontiguousarray(qf),
            "mf": mn,
            "syn": np.ascontiguousarray(syn[:, core * 128:(core + 1) * 128]),
            "coff": np.full((128, 1), float(core * SHARD), dtype=np.float32),
            "offs": np.broadcast_to(
                np.arange(64, dtype=np.float32) * 0.0625, (128, 64)).copy(),
        })
    return in_maps


_NC_CACHE = {}


def run(query_seq, matching_set, synth_set, topk=4, trace=False):
    assert int(topk) == 4, f"kernel is specialized for topk=4, got {topk}"
    in_maps = prepare_inputs(query_seq, matching_set, synth_set)
    if "nc" not in _NC_CACHE:
        _NC_CACHE["nc"] = build()
    nc = _NC_CACHE["nc"]
    res = run_bass_kernel_spmd(nc, in_maps, core_ids=list(range(NCORES)),
                               trace=trace)
    out = np.concatenate([res.results[i]["out"] for i in range(NCORES)], axis=1)
    return out.astype(np.float32), res


def kernel(**inputs):
    topk = inputs.get("topk", 4)
    try:
        topk = int(np.asarray(topk))
    except Exception:
        topk = int(topk)
    out, _ = run(inputs["query_seq"], inputs["matching_set"],
                 inputs["synth_set"], topk)
    return out


# revision 7
# speedup vs baseline: 3.0615x; 1.1296x over previous
"""Distributed kNN retrieval kernel for trn2 (8 NeuronCores) — v2.

Math: reference ranks candidates per query by cosine distance
1 - dot/(|q||m|); the query norm is constant per row, so ranking is by
dot(q, m)/|m|.  Host pre-normalizes matching rows (fp64 norms).

Two-stage select-then-rerank:
  stage 1 (approx): fp8-e4m3 DoubleRow matmul (2x bf16 rate) computes
    all 2048 x 12500 scores per core (candidates row-sharded).  Per-core
    top-8 per query via DVE max8/find_index8 on bf16 score rows.
  AllToAll: query-shards the 8x top-8 candidate lists (core c receives
    all peers' candidates for queries it owns).
  stage 2 (exact): merge to global top-16 per query, gather those 16
    matching rows (f32) by indirect DMA, recompute exact f32 dots on
    DVE, take top-4.
  AllGather final ids; every core gathers its 128 synth feature columns
  (synth feature-sharded) and averages; host concatenates features.

fp8 precision safety: e4m3 roundoff gives score-error sigma ~0.05 in
unit-score terms while the rank-4 -> rank-16 margin is ~0.34 (5 sigma),
so the true top-4 lies in the approx global top-16 w.h.p.; the exact
f32 rerank then reproduces the reference ranking.
"""
import sys

import numpy as np

sys.path.insert(0, "/opt/trn_rl_repo")
import ml_dtypes  # noqa: E402
import concourse.bacc as bacc  # noqa: E402
import concourse.bass as bass  # noqa: E402
import concourse.mybir as mybir  # noqa: E402
import concourse.tile as tile  # noqa: E402
from concourse.bass import IndirectOffsetOnAxis  # noqa: E402
from concourse.bass_utils import run_bass_kernel_spmd  # noqa: E402

NCORES = 8
FRM = 2048          # queries
F = 1024            # features
C = 100000          # candidates
SHARD = C // NCORES         # 12500
CW = 512                    # candidate-chunk width
NCCH = 25                   # chunks per core
SHARD_PAD = CW * NCCH       # 12800 (300 zero-pad rows)
NKP = 4                     # fp8 DoubleRow k-pairs (4 x 256 = 1024)
NQT = FRM // 128            # 16 query tiles
GQT = 2                     # qtiles per m8 sweep group
SQ = 16.0                   # fp8 scale for q
SM = 64.0                   # fp8 scale for normalized m
NSEL = 16                   # global rerank candidates per query

BF16 = mybir.dt.bfloat16
F32 = mybir.dt.float32
F8 = mybir.dt.float8e4
U16 = mybir.dt.uint16
U32 = mybir.dt.uint32
I32 = mybir.dt.int32
AOP = mybir.AluOpType


def build():
    nc = bacc.Bacc(num_devices=NCORES)
    Q8 = nc.declare_dram_parameter("q8", [128, NKP * 2 * FRM], F8, isOutput=False)
    M8 = nc.declare_dram_parameter("m8", [NCCH, 128, NKP * 2 * CW], F8, isOutput=False)
    QF = nc.declare_dram_parameter("qf", [256, F], F32, isOutput=False)
    MF = nc.declare_dram_parameter("mf", [C, F], F32, isOutput=False)
    SYN = nc.declare_dram_parameter("syn", [C, 128], F32, isOutput=False)
    COFF = nc.declare_dram_parameter("coff", [128, 1], F32, isOutput=False)
    OFFS = nc.declare_dram_parameter("offs", [128, 64], F32, isOutput=False)
    OUT = nc.declare_dram_parameter("out", [FRM, 128], F32, isOutput=True)

    with tile.TileContext(nc) as tc:
        with tc.tile_pool(name="cst", bufs=1) as cst, \
             tc.tile_pool(name="mpool", bufs=3) as mpool, \
             tc.tile_pool(name="spool", bufs=5) as spool, \
             tc.tile_pool(name="sm", bufs=8) as sm, \
             tc.tile_pool(name="rk", bufs=2) as rk, \
             tc.tile_pool(name="gat", bufs=3) as gat, \
             tc.tile_pool(name="sy", bufs=4) as sy, \
             tc.tile_pool(name="psw", bufs=1, space="PSUM") as psw, \
             tc.tile_pool(name="ps", bufs=4, space="PSUM") as ps, \
             tc.tile_pool(name="dram", bufs=2, space="DRAM") as dram:

            # ---- early loads + PE warmup ----
            q8t = cst.tile([128, NKP, 2, FRM], F8)
            nc.sync.dma_start(out=q8t[:, 0], in_=Q8[:, 0:2 * FRM])
            coff = cst.tile([128, 1], F32)
            nc.sync.dma_start(out=coff[:], in_=COFF[:])
            offs = cst.tile([128, 64], F32)
            nc.sync.dma_start(out=offs[:], in_=OFFS[:])

            pw = psw.tile([128, 128], F32)
            nc.tensor.matmul(out=pw[:], lhsT=q8t[:, 0, 0, 0:128],
                             rhs=q8t[:, 0, 0, 0:128], start=True, stop=True)
            for kp in range(1, NKP):
                nc.sync.dma_start(out=q8t[:, kp],
                                  in_=Q8[:, kp * 2 * FRM:(kp + 1) * 2 * FRM])

            at_in = [dram.tile([8 * 128, 16], F32, name=f"atin{h}", tag=f"atin{h}") for h in range(2)]
            at_out = [dram.tile([8 * 128, 16], F32, name=f"atout{h}", tag=f"atout{h}") for h in range(2)]
            ag_in = [dram.tile([128, 4], F32, name=f"agin{h}", tag=f"agin{h}") for h in range(2)]
            ag_out = [dram.tile([8 * 128, 4], F32, name=f"agout{h}", tag=f"agout{h}") for h in range(2)]

            def stage1_qtile_group(g):
                """Scores + per-core top-8 for qtiles 4g..4g+3."""
                srows = [spool.tile([128, SHARD_PAD], BF16, name=f"srow{g}_{i}", tag="srow")
                         for i in range(GQT)]
                for ch in range(NCCH):
                    mt = mpool.tile([128, NKP, 2, CW], F8, tag="mt")
                    nc.sync.dma_start(out=mt[:], in_=M8[ch])
                    for tl in range(GQT):
                        t = GQT * g + tl
                        p = ps.tile([128, CW], F32, tag="p")
                        for kp in range(NKP):
                            nc.tensor.matmul(
                                out=p[:],
                                lhsT=q8t[:, kp, :, t * 128:(t + 1) * 128],
                                rhs=mt[:, kp],
                                start=(kp == 0), stop=(kp == NKP - 1),
                                perf_mode=mybir.MatmulPerfMode.DoubleRow)
                        nc.scalar.copy(out=srows[tl][:, ch * CW:(ch + 1) * CW],
                                       in_=p[:])
                for tl in range(GQT):
                    t = GQT * g + tl
                    v8 = sm.tile([128, 8], BF16, tag="v8")
                    nc.vector.max(out=v8[:], in_=srows[tl][:])
                    i8 = sm.tile([128, 8], U16, tag="i8")
                    nc.vector.max_index(out=i8[:], in_max=v8[:], in_values=srows[tl][:])
                    pay = sm.tile([128, 16], F32, tag="pay")
                    nc.vector.tensor_copy(out=pay[:, 0:8], in_=v8[:])
                    i8f = sm.tile([128, 8], F32, tag="i8f")
                    nc.vector.tensor_copy(out=i8f[:], in_=i8[:])
                    nc.vector.tensor_scalar(
                        out=pay[:, 8:16], in0=i8f[:], scalar1=coff[:, 0:1],
                        scalar2=0.0, op0=AOP.add, op1=AOP.add)
                    h, tt = divmod(t, 8)
                    nc.sync.dma_start(out=at_in[h][tt * 128:(tt + 1) * 128, :],
                                      in_=pay[:])

            def rerank_block(h):
                """Merge + exact rerank for this core's qtile of half h."""
                cand = rk.tile([128, 8, 16], F32, tag="cand")
                nc.sync.dma_start(
                    out=cand[:],
                    in_=at_out[h][:].rearrange("(r p) e -> p r e", r=8))
                cv = rk.tile([128, 64], F32, tag="cv")
                ci = rk.tile([128, 64], F32, tag="ci")
                nc.vector.tensor_copy(
                    out=cv[:].rearrange("p (r e) -> p r e", r=8),
                    in_=cand[:, :, 0:8])
                nc.vector.tensor_copy(
                    out=ci[:].rearrange("p (r e) -> p r e", r=8),
                    in_=cand[:, :, 8:16])
                # de-tie: unique per-(core,slot) epsilon keeps the eq-trick
                # id resolution collision-free across bf16 score ties
                nc.vector.tensor_tensor(out=cv[:], in0=cv[:], in1=offs[:],
                                        op=AOP.add)
                gids = rk.tile([128, NSEL], F32, tag="gids")
                gv = rk.tile([128, 8], F32, tag="gv")
                cvs = cv
                for rnd in range(NSEL // 8):
                    nc.vector.max(out=gv[:], in_=cvs[:])
                    e3 = rk.tile([128, 8, 64], F32, tag="e3")
                    nc.vector.tensor_tensor(
                        out=e3[:],
                        in0=cvs[:].unsqueeze(1).to_broadcast([128, 8, 64]),
                        in1=gv[:].unsqueeze(2).to_broadcast([128, 8, 64]),
                        op=AOP.is_equal)
                    nc.vector.tensor_tensor(
                        out=e3[:], in0=e3[:],
                        in1=ci[:].unsqueeze(1).to_broadcast([128, 8, 64]),
                        op=AOP.mult)
                    nc.vector.tensor_reduce(
                        out=gids[:, rnd * 8:(rnd + 1) * 8], in_=e3[:],
                        axis=mybir.AxisListType.X, op=AOP.max)
                    if rnd == 0:
                        cv2 = rk.tile([128, 64], F32, tag="cv2")
                        nc.vector.match_replace(
                            out=cv2[:], in_to_replace=gv[:], in_values=cvs[:],
                            imm_value=-1e30)
                        cvs = cv2
                # clamp ids to [0, C-1] then cast to i32
                gcl = rk.tile([128, NSEL], F32, tag="gcl")
                nc.vector.tensor_scalar(
                    out=gcl[:], in0=gids[:], scalar1=float(C - 1), scalar2=0.0,
                    op0=AOP.min, op1=AOP.max)
                gii = rk.tile([128, NSEL], I32, tag="gii")
                nc.vector.tensor_copy(out=gii[:], in_=gcl[:])

                qft = rk.tile([128, F], F32, tag="qft")
                nc.sync.dma_start(out=qft[:], in_=QF[h * 128:(h + 1) * 128, :])
                rs = rk.tile([128, NSEL], F32, tag="rs")
                for j in range(NSEL):
                    gj = gat.tile([128, F], F32, tag="gj")
                    nc.gpsimd.indirect_dma_start(
                        out=gj[:], out_offset=None, in_=MF[:],
                        in_offset=IndirectOffsetOnAxis(ap=gii[:, j:j + 1], axis=0))
                    scr = gat.tile([128, F], F32, tag="scr")
                    nc.vector.scalar_tensor_tensor(
                        out=scr[:], in0=qft[:], scalar=1.0, in1=gj[:],
                        op0=AOP.mult, op1=AOP.mult, accum_out=rs[:, j:j + 1])
                fv = rk.tile([128, 8], F32, tag="fv")
                nc.vector.max(out=fv[:], in_=rs[:])
                e4 = rk.tile([128, 4, NSEL], F32, tag="e4")
                nc.vector.tensor_tensor(
                    out=e4[:],
                    in0=rs[:].unsqueeze(1).to_broadcast([128, 4, NSEL]),
                    in1=fv[:, 0:4].unsqueeze(2).to_broadcast([128, 4, NSEL]),
                    op=AOP.is_equal)
                nc.vector.tensor_tensor(
                    out=e4[:], in0=e4[:],
                    in1=gids[:].unsqueeze(1).to_broadcast([128, 4, NSEL]),
                    op=AOP.mult)
                fi = rk.tile([128, 4], F32, tag="fi")
                nc.vector.tensor_reduce(
                    out=fi[:], in_=e4[:], axis=mybir.AxisListType.X, op=AOP.max)
                nc.sync.dma_start(out=ag_in[h][:], in_=fi[:])

            def synth_block(h):
                """Gather-mean of this core's 128 synth columns, qtiles 8h..8h+7."""
                for r in range(8):
                    t = 8 * h + r
                    idt = sy.tile([128, 4], F32, tag="idt")
                    nc.sync.dma_start(out=idt[:],
                                      in_=ag_out[h][r * 128:(r + 1) * 128, :])
                    idc = sy.tile([128, 4], F32, tag="idc")
                    nc.vector.tensor_scalar(
                        out=idc[:], in0=idt[:], scalar1=float(C - 1), scalar2=0.0,
                        op0=AOP.min, op1=AOP.max)
                    idi = sy.tile([128, 4], I32, tag="idi")
                    nc.vector.tensor_copy(out=idi[:], in_=idc[:])
                    gb = sy.tile([128, 128], F32, tag="gb")
                    nc.vector.memset(gb[:], 0.0)
                    for j in range(4):
                        nc.gpsimd.indirect_dma_start(
                            out=gb[:], out_offset=None, in_=SYN[:],
                            in_offset=IndirectOffsetOnAxis(ap=idi[:, j:j + 1], axis=0),
                            compute_op=AOP.add)
                    nc.vector.tensor_scalar_mul(gb[:], gb[:], 0.25)
                    nc.sync.dma_start(out=OUT[t * 128:(t + 1) * 128, :], in_=gb[:])

            rg = [list(range(NCORES))]
            NG = NQT // GQT
            # ---- half 1: qtiles 0..7 ----
            for g in range(NG // 2):
                stage1_qtile_group(g)
            nc.gpsimd.collective_compute(
                "AllToAll", AOP.bypass, replica_groups=rg,
                ins=[at_in[0].opt()], outs=[at_out[0].opt()])
            rerank_block(0)
            nc.gpsimd.collective_compute(
                "AllGather", AOP.bypass, replica_groups=rg,
                ins=[ag_in[0].opt()], outs=[ag_out[0].opt()])
            # ---- half 2: qtiles 8..15 (overlaps with rerank/synth of half 1) ----
            for g in range(NG // 2, NG):
                stage1_qtile_group(g)
            synth_block(0)
            nc.gpsimd.collective_compute(
                "AllToAll", AOP.bypass, replica_groups=rg,
                ins=[at_in[1].opt()], outs=[at_out[1].opt()])
            rerank_block(1)
            nc.gpsimd.collective_compute(
                "AllGather", AOP.bypass, replica_groups=rg,
                ins=[ag_in[1].opt()], outs=[ag_out[1].opt()])
            synth_block(1)

    nc.compile()
    return nc


# ---------------- host side ----------------

def _to_fp8(x):
    return np.clip(x, -240.0, 240.0).astype(ml_dtypes.float8_e4m3)


def prepare_inputs(query_seq, matching_set, synth_set):
    q = np.asarray(query_seq, dtype=np.float32)
    m = np.asarray(matching_set, dtype=np.float32)
    syn = np.asarray(synth_set, dtype=np.float32)

    norms = np.linalg.norm(m.astype(np.float64), axis=1, keepdims=True)
    mn = (m / norms).astype(np.float32)

    # q8: [1024, 2048] -> [kp 4, ko 2, p 128, col 2048] -> [p, kp, ko, col]
    qt = np.ascontiguousarray(q.T) * SQ
    q8 = _to_fp8(qt).reshape(NKP, 2, 128, FRM).transpose(2, 0, 1, 3)
    q8 = np.ascontiguousarray(q8).reshape(128, NKP * 2 * FRM)

    in_maps = []
    for core in range(NCORES):
        shard = mn[core * SHARD:(core + 1) * SHARD] * SM    # [12500, 1024]
        sp = np.zeros((SHARD_PAD, F), dtype=np.float32)
        sp[:SHARD] = shard
        # [1024, 12800] -> [kp, ko, p, ch, cw] -> [ch, p, kp, ko, cw]
        mt8 = _to_fp8(sp.T).reshape(NKP, 2, 128, NCCH, CW).transpose(3, 2, 0, 1, 4)
        mt8 = np.ascontiguousarray(mt8).reshape(NCCH, 128, NKP * 2 * CW)

        qf = np.concatenate([q[core * 128:(core + 1) * 128],
                             q[1024 + core * 128:1024 + (core + 1) * 128]], axis=0)

        in_maps.append({
            "q8": q8,
            "m8": mt8,
            "qf": np.ascontiguousarray(qf),
            "mf": mn,
            "syn": np.ascontiguousarray(syn[:, core * 128:(core + 1) * 128]),
            "coff": np.full((128, 1), float(core * SHARD), dtype=np.float32),
            "offs": np.broadcast_to(
                np.arange(64, dtype=np.float32) * 0.0625, (128, 64)).copy(),
        })
    return in_maps


_NC_CACHE = {}


def run(query_seq, matching_set, synth_set, topk=4, trace=False):
    assert int(topk) == 4, f"kernel is specialized for topk=4, got {topk}"
    in_maps = prepare_inputs(query_seq, matching_set, synth_set)
    if "nc" not in _NC_CACHE:
        _NC_CACHE["nc"] = build()
    nc = _NC_CACHE["nc"]
    res = run_bass_kernel_spmd(nc, in_maps, core_ids=list(range(NCORES)),
                               trace=trace)
    out = np.concatenate([res.results[i]["out"] for i in range(NCORES)], axis=1)
    return out.astype(np.float32), res


def kernel(**inputs):
    topk = inputs.get("topk", 4)
    try:
        topk = int(np.asarray(topk))
    except Exception:
        topk = int(topk)
    out, _ = run(inputs["query_seq"], inputs["matching_set"],
                 inputs["synth_set"], topk)
    return out
